# revision 1
# baseline (speedup 1.0000x reference)
"""Trainium2 Bass kernel for nn_BinaryBlock (binary 3x3 conv block).

Reference semantics (forward values only):
    z   = prelu(x + bias1) + bias2          (per-channel prelu slope a)
    act = sign(z)                           (binary activation, +-1)
    bw  = sf[o] * sign(w)                   (sf = per-out-channel mean|w|)
    y   = conv3x3(act, bw, pad=1)
        + grouped_pool(x)                   (out o: pw[o,0]*x[2o]+pw[o,1]*x[2o+1])
    y   = pixel_unshuffle(y, 2)             (B,64,128,128) -> (B,256,64,64)

Kernel strategy (8 NeuronCores, data-parallel over batch, 2 images/core):
  * prelu chain is monotonic (a>0) so act = sign(x - t[c]); t computed on host
    and applied as the ScalarE Sign activation's per-partition bias, writing
    fp8e4 +-1 activations directly (exact).
  * conv runs as shifted matmuls (K=C=128, M=CO=64, N=512 = 4 rows x 128 cols)
    accumulating in PSUM.  fp8 +-1 products are exact and sums (<=1152) are
    exact in fp32.  The (di=-1, di=0) tap pairs of each dj column ride ONE
    fp8 DoubleRow matmul (2 K-rows/cell; the act row stride of 144 satisfies
    the DoubleRow step%16==0 constraint), so each 4-row tile x image needs
    only 3 DoubleRow + 3 solo conv matmuls + 1 shortcut matmul.
  * the grouped 1x1 shortcut is an fp16 matmul with weights pool_w/sf against
    fp16-cast x, accumulated into the same PSUM bank (fp16 keeps its error
    ~2^-11 relative, well under tolerance).
  * per PSUM bank: partitions 0:64 = img0, 64:128 = img1 (PE column groups
    h0/h64).  This makes the output DMA partition stride constant
    (img stride 256*64*64 == 64 * channel-block stride).
  * final pass: one tensor_scalar per output parity pair multiplies by sf[o]
    while scattering PSUM into the pixel-unshuffled layout; one DMA per band
    stores it.
"""

import sys

import numpy as np

try:
    import concourse.bass as bass  # noqa: F401
except ImportError:  # pragma: no cover
    sys.path.insert(0, "/opt/trn_rl_repo")
    import concourse.bass as bass

import concourse.mybir as mybir
from concourse import bacc
from concourse.bass_utils import run_bass_kernel_spmd
from concourse.tile import TileContext

N_CORES = 8
B, C, H, W = 16, 128, 128, 128
CO = C // 2
BPC = B // N_CORES  # images per core
BAND_ROWS = 32
BANDS = H // BAND_ROWS
NT = BAND_ROWS // 4  # 4-row tiles per band
AW = 144  # act row stride (>= W+2, multiple of 16 for DoubleRow Ko step)

_nc_cache = None


def _dr_rhs(act, rs, dj):
    """DoubleRow moving operand: [K=128, Ko=2, rows=4, cols=128] where the
    Ko dim steps one act row (di=-1 -> di=0).  Built by prepending a
    [step=AW, count=2] dim to the plain 3D slice's access pattern."""
    base = act[:, rs : rs + 4, dj + 1 : dj + 1 + W]
    ap = [list(d) for d in base.ap]
    ap.insert(1, [AW, 2])
    return bass.AP(base.tensor, base.offset, ap)


def build_nc(reps=1):
    """reps>1 wraps the whole body in a hardware For_i loop (timing only)."""
    f32 = mybir.dt.float32
    f16 = mybir.dt.float16
    fp8 = mybir.dt.float8e4

    nc = bacc.Bacc()
    x_d = nc.dram_tensor("x", [BPC, C, H, W], f32, kind="ExternalInput")
    wtp_d = nc.dram_tensor("wtp", [C, 3, 2, CO], fp8, kind="ExternalInput")
    wts_d = nc.dram_tensor("wts", [C, 3, CO], fp8, kind="ExternalInput")
    pw_d = nc.dram_tensor("pw", [C, CO], f16, kind="ExternalInput")
    thr_d = nc.dram_tensor("thr", [C, 1], f32, kind="ExternalInput")
    sf_d = nc.dram_tensor("sf", [C, 1], f32, kind="ExternalInput")
    y_d = nc.dram_tensor("y", [BPC, 4 * CO, H // 2, W // 2], f32, kind="ExternalOutput")
    # DMA view: [(b o)=128, ij=4, ho=64, wo=64]; merging (b o) is valid because
    # the image stride (256*64*64) equals 64x the channel-block stride (4*64*64).
    y_r = y_d.rearrange("b (o ij) h w -> (b o) ij h w", ij=4)

    with TileContext(nc) as tc:
        with (
            tc.tile_pool(name="cpool", bufs=1) as cpool,
            tc.tile_pool(name="spool", bufs=3) as spool,
            tc.tile_pool(name="apool", bufs=4) as apool,
            tc.tile_pool(name="hpool", bufs=4) as hpool,
            tc.tile_pool(name="opool", bufs=2) as opool,
            tc.tile_pool(name="pspool", bufs=8, space="PSUM") as pspool,
        ):
            wtp = cpool.tile([C, 3, 2, CO], fp8)
            nc.sync.dma_start(out=wtp, in_=wtp_d[:, :, :, :])
            wts = cpool.tile([C, 3, CO], fp8)
            nc.sync.dma_start(out=wts, in_=wts_d[:, :, :])
            pw = cpool.tile([C, CO], f16)
            nc.sync.dma_start(out=pw, in_=pw_d[:, :])
            thr = cpool.tile([C, 1], f32)
            nc.sync.dma_start(out=thr, in_=thr_d[:, :])
            sfv = cpool.tile([C, 1], f32)
            nc.sync.dma_start(out=sfv, in_=sf_d[:, :])

            consts = (wtp, wts, pw, thr, sfv)

            def body():
                run_bands(nc, x_d, y_r, consts, spool, apool, hpool, opool, pspool)

            if reps == 1:
                body()
            else:
                with tc.For_i(0, reps, 1):
                    body()
    nc.finalize()
    return nc


def run_bands(nc, x_d, y_r, consts, spool, apool, hpool, opool, pspool):
    f32 = mybir.dt.float32
    f16 = mybir.dt.float16
    fp8 = mybir.dt.float8e4
    wtp, wts, pw, thr, sfv = consts

    for band in range(BANDS):
        r0 = band * BAND_ROWS
        lo = max(r0 - 1, 0)
        hi = min(r0 + BAND_ROWS + 1, H)
        nrows = hi - lo
        acts, xhis = [], []
        for img in range(BPC):
            xs = spool.tile(
                [C, BAND_ROWS + 2, W], f32, tag="stage", name=f"xs_{band}_{img}"
            )
            nc.sync.dma_start(out=xs[:, :nrows, :], in_=x_d[img, :, lo:hi, :])
            # padded binary activations: rows = act rows r0-1..r0+32,
            # cols 1..W hold act cols 0..W-1; pad cols 0 and W+1 are zero.
            # Cols W+2..AW-1 are never read.
            act = apool.tile(
                [C, BAND_ROWS + 2, AW], fp8, tag="act", name=f"act_{band}_{img}"
            )
            nc.vector.memset(act[:, :, 0:1], 0.0)
            nc.vector.memset(act[:, :, W + 1 : W + 2], 0.0)
            row0 = 0
            if band == 0:
                nc.vector.memset(act[:, 0:1, : W + 2], 0.0)
                row0 = 1
            if band == BANDS - 1:
                nc.vector.memset(act[:, BAND_ROWS + 1 : BAND_ROWS + 2, : W + 2], 0.0)
            nc.scalar.sign(
                act[:, row0 : row0 + nrows, 1 : W + 1],
                xs[:, :nrows, :],
                bias=thr[:, 0:1],
            )
            xh = hpool.tile([C, BAND_ROWS, W], f16, tag="hi", name=f"xh_{band}_{img}")
            off = r0 - lo
            nc.vector.tensor_copy(out=xh, in_=xs[:, off : off + BAND_ROWS, :])
            acts.append(act)
            xhis.append(xh)

        outts = [
            opool.tile(
                [CO, 4, 2 * NT, W // 2], f32, tag=f"out{img}", name=f"outt_{band}_{img}"
            )
            for img in range(BPC)
        ]
        for rt in range(NT):
            for img in range(BPC):
                # DoubleRow matmuls require dst partition base 0, so each
                # (rt, img) group gets its own PSUM bank at partitions 0:64.
                ps = pspool.tile(
                    [CO, 4, W], f32, tag="ps", name=f"ps_{band}_{rt}_{img}"
                )
                act = acts[img]
                # 7 matmuls per group: 3 DoubleRow (di=-1&0 per dj), 3 solo
                # (di=+1 per dj), 1 fp16 shortcut.  All self-loading; the
                # per-matmul LDWEIGHTS hides behind the 512-cycle matmuls.
                for mi in range(7):
                    if mi < 3:
                        dj = mi
                        lhsT = wtp[:, dj, :, :]
                        rhs = _dr_rhs(act, 4 * rt, dj - 1)
                        pm = mybir.MatmulPerfMode.DoubleRow
                    elif mi < 6:
                        dj = mi - 3
                        lhsT = wts[:, dj, :]
                        rhs = act[:, 4 * rt + 2 : 4 * rt + 6, dj : dj + W]
                        pm = None
                    else:
                        lhsT = pw[:, :]
                        rhs = xhis[img][:, 4 * rt : 4 * rt + 4, :]
                        pm = None
                    nc.tensor.matmul(
                        ps[:, :, :],
                        lhsT,
                        rhs,
                        start=mi == 0,
                        stop=mi == 6,
                        perf_mode=pm,
                    )
                # pixel-unshuffle + sf scale: psum[p, (r i), (c j)] ->
                # out[p, i*2+j, 2*rt + r, c]
                psv = ps.rearrange("p (r i) (c j) -> p r i c j", i=2, j=2)
                for i in range(2):
                    for j in range(2):
                        nc.vector.tensor_scalar_mul(
                            outts[img][:, 2 * i + j, 2 * rt : 2 * rt + 2, :],
                            psv[:, :, i, :, j],
                            sfv[:CO, 0:1],
                        )
        for img in range(BPC):
            nc.sync.dma_start(
                out=y_r[
                    img * CO : (img + 1) * CO,
                    :,
                    band * 2 * NT : (band + 1) * 2 * NT,
                    :,
                ],
                in_=outts[img],
            )


def prep_params(bias1, prelu_a, bias2, conv_w, pool_w):
    """Host-side folding of the tiny parameter tensors."""
    fp8np = mybir.dt.np(mybir.dt.float8e4)
    b1 = np.asarray(bias1, np.float64).reshape(C)
    a = np.asarray(prelu_a, np.float64).reshape(C)
    b2 = np.asarray(bias2, np.float64).reshape(C)
    if not np.all(a > 0):
        raise NotImplementedError("kernel assumes strictly positive PReLU slope")
    # z(x) = prelu(x + b1) + b2 is strictly increasing; z = 0 at x = t.
    u0 = np.where(-b2 >= 0, -b2, -b2 / a)
    t = u0 - b1
    thr = (-t).astype(np.float32).reshape(C, 1)  # sign(x + thr) == sign(z)

    w = np.asarray(conv_w, np.float32).reshape(CO, C, 3, 3)
    sf = np.mean(np.abs(w), axis=(1, 2, 3), dtype=np.float32)  # [CO]
    wsign = np.sign(w).astype(np.float32)  # [CO, C, kh, kw]; kh = di+1, kw = dj+1
    # DoubleRow pairs: Ko=0 -> di=-1 (kh=0), Ko=1 -> di=0 (kh=1), per dj.
    wtp = (
        np.transpose(wsign[:, :, 0:2, :], (1, 3, 2, 0))  # [C, dj, Ko, CO]
        .astype(fp8np)
        .copy()
    )
    # solo taps: di=+1 (kh=2)
    wts = np.transpose(wsign[:, :, 2, :], (1, 2, 0)).astype(fp8np).copy()

    pwf = np.asarray(pool_w, np.float64).reshape(CO, 2)
    pwm = np.zeros((C, CO), np.float64)
    o = np.arange(CO)
    pwm[2 * o, o] = pwf[:, 0] / sf
    pwm[2 * o + 1, o] = pwf[:, 1] / sf
    pw16 = pwm.astype(np.float16)

    sfd = np.concatenate([sf, sf]).astype(np.float32).reshape(C, 1)
    return thr, wtp, wts, pw16, sfd


def make_in_maps(x, bias1, prelu_a, bias2, conv_w, pool_w):
    thr, wtp, wts, pw16, sfd = prep_params(bias1, prelu_a, bias2, conv_w, pool_w)
    x = np.ascontiguousarray(np.asarray(x, np.float32))
    assert x.shape == (B, C, H, W), x.shape
    return [
        {
            "x": x[i * BPC : (i + 1) * BPC],
            "wtp": wtp,
            "wts": wts,
            "pw": pw16,
            "thr": thr,
            "sf": sfd,
        }
        for i in range(N_CORES)
    ]


def kernel(x, bias1, prelu_a, bias2, conv_w, pool_w):
    global _nc_cache
    in_maps = make_in_maps(x, bias1, prelu_a, bias2, conv_w, pool_w)
    if _nc_cache is None:
        _nc_cache = build_nc()
    res = run_bass_kernel_spmd(_nc_cache, in_maps, list(range(N_CORES)))
    y = np.concatenate([res.results[i]["y"] for i in range(N_CORES)], axis=0)
    return np.ascontiguousarray(y.astype(np.float32))

